# revision 2
# baseline (speedup 1.0000x reference)
"""DiffAttn kernel for 8 trn2 NeuronCores (v2: bf16 dataflow).

Problem (per reference):
  X [4, 4096, 1024]; Wq/Wk [1024, 256]; Wv [1024, 128]; biases; lam scalar.
  Q,K = X@Wq+bq, X@Wk+bk ; V = X@Wv+bv
  A_i = Q_i @ K_i^T / sqrt(128)  (i = 1,2 : the two 128-wide halves)
  out = (softmax(A1) - lam * softmax(A2)) @ V          -> [4, 4096, 128]

Sharding: 8 cores = 4 batches x 2 query-halves. Each core computes the
attention output for 2048 queries of one batch; K/V projections for the
full 4096 keys of that batch are computed redundantly on both cores of the
pair (no collectives). Host passes X^T per core (bf16) with the core's
query rows ordered first; key order is irrelevant to softmax.

v2 dataflow (all matmul operands bf16, PSUM accumulation fp32):
  Projections: X^T streamed in 512-col chunks; Q/K into head-transposed
  [128, S] bf16; V via PE transpose into [key, d] bf16.
  Attention per (super=1024 queries, key-tile): both components' scores
  land in ONE [128, 2x1024] fp32 PSUM tile -> a single FD=2048 exp
  ACTIVATE (1/sqrt(D) folded into the activation scale) writes e12 bf16;
  DVE accumulates softmax denominators in bf16 at 2x; PV accumulates in
  PSUM over the 32 key tiles. Finalize: ones-matmul broadcasts the
  partition-sums of pacc to all 128 partitions, ScalarE evacuates, DVE
  reciprocal_approx_fast + 3 tensor ops; lam is folded into the o2
  evacuation scale. Output ships as O^T [128, 2048] fp32.
"""

import sys

sys.path.insert(0, "/opt/trn_rl_repo")

import numpy as np

import concourse.bacc as bacc
import concourse.mybir as mybir
from concourse import masks
from concourse.tile import TileContext
from concourse.bass_utils import run_bass_kernel_spmd

F32 = mybir.dt.float32
BF16 = mybir.dt.bfloat16
AF = mybir.ActivationFunctionType

D = 128
EMB = 1024
B, S = 4, 4096
NQ = S // 2          # queries per core
SQC = 512            # projection column chunk
NCC = S // SQC       # 8 projection column chunks
NE = EMB // 128      # 8 emb k-tiles
SUP = 1024           # attention query super-chunk
NSUP = NQ // SUP     # 2
NSK = S // 128       # 32 key tiles
INV_SQRT_D = 1.0 / np.sqrt(np.float32(D))

TRACE = False
TRACE_DIR = None
LAST_RESULT = None


def _build():
    nc = bacc.Bacc("TRN2", target_bir_lowering=False, debug=False, num_devices=8)

    xt = nc.dram_tensor("xt", [EMB, S], BF16, kind="ExternalInput")
    wq = nc.dram_tensor("wq", [EMB, 2 * D], BF16, kind="ExternalInput")
    wk = nc.dram_tensor("wk", [EMB, 2 * D], BF16, kind="ExternalInput")
    wv = nc.dram_tensor("wv", [EMB, D], BF16, kind="ExternalInput")
    bq = nc.dram_tensor("bq", [2 * D, 1], F32, kind="ExternalInput")
    bk = nc.dram_tensor("bk", [2 * D, 1], F32, kind="ExternalInput")
    bv = nc.dram_tensor("bv", [D, 1], F32, kind="ExternalInput")
    lamv = nc.dram_tensor("lamv", [128, 1], F32, kind="ExternalInput")
    out = nc.dram_tensor("o", [D, NQ], F32, kind="ExternalOutput")  # O^T

    from contextlib import ExitStack

    with TileContext(nc) as tc, ExitStack() as ctx:
        # ---- DMAs: weights for the first group + first chunks early ----
        wpool = ctx.enter_context(tc.tile_pool(name="w", bufs=1))
        wk1 = wpool.tile([128, NE, 128], BF16, tag="wk1")
        wk2 = wpool.tile([128, NE, 128], BF16, tag="wk2")
        wq1 = wpool.tile([128, NE, 128], BF16, tag="wq1")
        wq2 = wpool.tile([128, NE, 128], BF16, tag="wq2")
        wvt = wpool.tile([128, NE, 128], BF16, tag="wvt")

        xpool = ctx.enter_context(tc.tile_pool(name="xt", bufs=1))
        xt_all = xpool.tile([128, NCC, NE, SQC], BF16, tag="xall")

        def wsrc(w, dsl):
            return w[:, dsl].rearrange("(t p) d -> p t d", p=128)

        def load_chunk(cc):
            nc.sync.dma_start(
                out=xt_all[:, cc],
                in_=xt[:, cc * SQC : (cc + 1) * SQC].rearrange(
                    "(t p) s -> p t s", p=128
                ),
            )

        nc.sync.dma_start(out=wk1[:], in_=wsrc(wk, slice(0, 128)))
        load_chunk(0)
        load_chunk(1)
        nc.sync.dma_start(out=wk2[:], in_=wsrc(wk, slice(128, 256)))
        nc.sync.dma_start(out=wvt[:], in_=wsrc(wv, slice(0, 128)))
        nc.sync.dma_start(out=wq1[:], in_=wsrc(wq, slice(0, 128)))
        nc.sync.dma_start(out=wq2[:], in_=wsrc(wq, slice(128, 256)))
        for cc in range(2, NCC):
            load_chunk(cc)

        cpool = ctx.enter_context(tc.tile_pool(name="const", bufs=1))
        ident = cpool.tile([128, 128], BF16, tag="ident")
        masks.make_identity(nc, ident[:])
        ones_sq = cpool.tile([128, 128], BF16, tag="ones_sq")
        nc.vector.memset(ones_sq[:], 1.0)

        bq1 = cpool.tile([128, 1], F32, tag="bq1")
        bq2 = cpool.tile([128, 1], F32, tag="bq2")
        bk1 = cpool.tile([128, 1], F32, tag="bk1")
        bk2 = cpool.tile([128, 1], F32, tag="bk2")
        bvt = cpool.tile([128, 1], F32, tag="bvt")
        lam_t = cpool.tile([128, 1], F32, tag="lam")
        nc.gpsimd.dma_start(out=bq1[:], in_=bq[0:128, :])
        nc.gpsimd.dma_start(out=bq2[:], in_=bq[128:256, :])
        nc.gpsimd.dma_start(out=bk1[:], in_=bk[0:128, :])
        nc.gpsimd.dma_start(out=bk2[:], in_=bk[128:256, :])
        nc.gpsimd.dma_start(out=bvt[:], in_=bv[0:128, :])
        nc.gpsimd.dma_start(out=lam_t[:], in_=lamv[:, :])

        qkv = ctx.enter_context(tc.tile_pool(name="qkv", bufs=1))
        qt1 = qkv.tile([128, NQ], BF16, tag="qt1")
        qt2 = qkv.tile([128, NQ], BF16, tag="qt2")
        kt1 = qkv.tile([128, S], BF16, tag="kt1")
        kt2 = qkv.tile([128, S], BF16, tag="kt2")
        vv = qkv.tile([128, S], BF16, tag="vv")  # [key%128, kt*128+d] = V[key, d]

        epool = ctx.enter_context(tc.tile_pool(name="e", bufs=3))
        papool = ctx.enter_context(tc.tile_pool(name="pacc", bufs=2))
        fpool = ctx.enter_context(tc.tile_pool(name="fin", bufs=2))

        # ---------------- projections ----------------
        with ExitStack() as pctx:
            ppool = pctx.enter_context(tc.tile_pool(name="ppsum", bufs=1, space="PSUM"))
            tpool = pctx.enter_context(tc.tile_pool(name="ptr", bufs=2, space="PSUM"))
            vspool = pctx.enter_context(tc.tile_pool(name="vts", bufs=2))

            def proj_group(dst, w_t, b_t, chunks, tag, chunk_outer):
                ps = ppool.tile([128, 4, SQC], F32, tag="proj", name=f"ps_{tag}")
                if chunk_outer:
                    # first group: start as soon as chunk 0 lands
                    for ci, c in enumerate(chunks):
                        for e in range(NE):
                            nc.tensor.matmul(
                                ps[:, ci, :],
                                w_t[:, e, :],
                                xt_all[:, c, e, :],
                                start=(e == 0),
                                stop=(e == NE - 1),
                            )
                else:
                    # weight-stationary order: each LDW feeds 4 matmuls
                    for e in range(NE):
                        for ci, c in enumerate(chunks):
                            nc.tensor.matmul(
                                ps[:, ci, :],
                                w_t[:, e, :],
                                xt_all[:, c, e, :],
                                start=(e == 0),
                                stop=(e == NE - 1),
                            )
                for ci, c in enumerate(chunks):
                    csl = slice(c * SQC, (c + 1) * SQC)
                    if dst is not None:
                        nc.scalar.activation(
                            dst[:, csl], ps[:, ci, :], AF.Identity, bias=b_t[:, 0:1]
                        )
                    else:
                        vt_s = vspool.tile([128, SQC], BF16, tag="vts", name=f"vt_{c}")
                        nc.scalar.activation(
                            vt_s[:], ps[:, ci, :], AF.Identity, bias=b_t[:, 0:1]
                        )
                        for j in range(SQC // 128):
                            tr = tpool.tile(
                                [128, 128], BF16, tag="vtr", name=f"vtr_{c}_{j}"
                            )
                            nc.tensor.transpose(
                                tr[:], vt_s[:, j * 128 : (j + 1) * 128], ident[:]
                            )
                            col = (c * (SQC // 128) + j) * 128
                            nc.vector.tensor_copy(vv[:, col : col + 128], tr[:])

            for half in range(2):
                chunks = list(range(half * 4, half * 4 + 4))
                proj_group(kt1, wk1, bk1, chunks, f"k1_{half}", half == 0)
                proj_group(kt2, wk2, bk2, chunks, f"k2_{half}", False)
                proj_group(None, wvt, bvt, chunks, f"v_{half}", False)
                if half == 0:
                    proj_group(qt1, wq1, bq1, chunks, "q1", False)
                    proj_group(qt2, wq2, bq2, chunks, "q2", False)

        # ---------------- attention ----------------
        with ExitStack() as actx:
            spool = actx.enter_context(tc.tile_pool(name="spsum", bufs=1, space="PSUM"))
            opool = actx.enter_context(tc.tile_pool(name="opsum", bufs=1, space="PSUM"))

            for sup in range(NSUP):
                qa = sup * SUP
                o1 = opool.tile([128, SUP], F32, tag="o1", name=f"o1_{sup}")
                o2 = opool.tile([128, SUP], F32, tag="o2", name=f"o2_{sup}")
                pacc = papool.tile([128, 2, SUP], BF16, tag="pacc", name=f"pa_{sup}")

                for kt in range(NSK):
                    ksl = slice(kt * 128, (kt + 1) * 128)
                    s12 = spool.tile(
                        [128, 2, SUP], F32, tag="s12", name=f"s12_{sup}_{kt}"
                    )
                    for comp, (ktc, qtc) in enumerate([(kt1, qt1), (kt2, qt2)]):
                        for h in range(SUP // 512):
                            nc.tensor.matmul(
                                s12[:, comp, h * 512 : (h + 1) * 512],
                                ktc[:, ksl],
                                qtc[:, qa + h * 512 : qa + (h + 1) * 512],
                                start=True,
                                stop=True,
                            )
                    e12 = epool.tile(
                        [128, 2, SUP], BF16, tag="e12", name=f"e_{sup}_{kt}"
                    )
                    nc.scalar.activation(
                        e12[:], s12[:], AF.Exp, scale=float(INV_SQRT_D)
                    )
                    if kt == 0:
                        nc.vector.tensor_copy(pacc[:], e12[:])
                    else:
                        nc.vector.tensor_add(pacc[:], pacc[:], e12[:])
                    for comp, o_ps in enumerate([o1, o2]):
                        for h in range(SUP // 512):
                            nc.tensor.matmul(
                                o_ps[:, h * 512 : (h + 1) * 512],
                                vv[:, ksl],
                                e12[:, comp, h * 512 : (h + 1) * 512],
                                start=(kt == 0),
                                stop=(kt == NSK - 1),
                            )

                # ---- finalize this super-chunk ----
                # broadcast partition-sums of pacc to all 128 partitions
                rs_b = spool.tile([128, 2, SUP], F32, tag="s12", name=f"rsb_{sup}")
                for comp in range(2):
                    for h in range(SUP // 512):
                        nc.tensor.matmul(
                            rs_b[:, comp, h * 512 : (h + 1) * 512],
                            ones_sq[:],
                            pacc[:, comp, h * 512 : (h + 1) * 512],
                            start=True,
                            stop=True,
                        )
                rsr = fpool.tile([128, 2, SUP], F32, tag="rsr", name=f"rsr_{sup}")
                nc.scalar.activation(rsr[:], rs_b[:], AF.Identity)
                o1s = fpool.tile([128, SUP], BF16, tag="o1s", name=f"o1s_{sup}")
                nc.scalar.activation(o1s[:], o1[:], AF.Identity)
                o2s = fpool.tile([128, SUP], BF16, tag="o2s", name=f"o2s_{sup}")
                # fold lam into the o2 evacuation
                nc.scalar.activation(o2s[:], o2[:], AF.Identity, scale=lam_t[:, 0:1])

                ir = fpool.tile([128, 2, SUP], F32, tag="ir", name=f"ir_{sup}")
                nc.vector.reciprocal_approx_fast(ir[:], rsr[:])
                t1 = fpool.tile([128, SUP], F32, tag="t1", name=f"t1_{sup}")
                nc.vector.tensor_mul(t1[:], o1s[:], ir[:, 0, :])
                t2 = fpool.tile([128, SUP], F32, tag="t2", name=f"t2_{sup}")
                nc.vector.tensor_mul(t2[:], o2s[:], ir[:, 1, :])
                o_t = fpool.tile([128, SUP], F32, tag="ot", name=f"ot_{sup}")
                nc.vector.tensor_sub(o_t[:], t1[:], t2[:])
                nc.sync.dma_start(out=out[:, qa : qa + SUP], in_=o_t[:])

    nc.compile()
    return nc


_NC = None


def _get_nc():
    global _NC
    if _NC is None:
        _NC = _build()
    return _NC


def kernel(X, lam, Wq, bq, Wk, bk, Wv, bv):
    import ml_dtypes

    BF = ml_dtypes.bfloat16
    X = np.asarray(X, dtype=np.float32)
    lam_f = float(np.asarray(lam))
    Wq_b = np.ascontiguousarray(np.asarray(Wq, np.float32).astype(BF))
    Wk_b = np.ascontiguousarray(np.asarray(Wk, np.float32).astype(BF))
    Wv_b = np.ascontiguousarray(np.asarray(Wv, np.float32).astype(BF))
    bq_c = np.asarray(bq, np.float32).reshape(2 * D, 1).copy()
    bk_c = np.asarray(bk, np.float32).reshape(2 * D, 1).copy()
    bv_c = np.asarray(bv, np.float32).reshape(D, 1).copy()
    lam_v = np.full((128, 1), lam_f, np.float32)

    nc = _get_nc()

    in_maps = []
    for core in range(8):
        b, h = divmod(core, 2)
        xb = X[b]
        if h == 0:
            xr = xb
        else:
            xr = np.concatenate([xb[NQ:], xb[:NQ]], axis=0)
        xt_a = np.ascontiguousarray(xr.T.astype(BF))
        in_maps.append(
            {
                "xt": xt_a,
                "wq": Wq_b,
                "wk": Wk_b,
                "wv": Wv_b,
                "bq": bq_c,
                "bk": bk_c,
                "bv": bv_c,
                "lamv": lam_v,
            }
        )

    global LAST_RESULT
    kwargs = {}
    if TRACE:
        import tempfile

        tdir = tempfile.mkdtemp(dir=TRACE_DIR) if TRACE_DIR else None
        kwargs = dict(trace=True, tmpdir=tdir)
    res = run_bass_kernel_spmd(nc, in_maps, list(range(8)), **kwargs)
    LAST_RESULT = res

    o = np.empty((B, S, D), np.float32)
    for core in range(8):
        b, h = divmod(core, 2)
        o[b, h * NQ : (h + 1) * NQ, :] = np.asarray(
            res.results[core]["o"], np.float32
        ).T
    return o


# revision 3
# speedup vs baseline: 1.9401x; 1.9401x over previous
"""DiffAttn kernel for 8 trn2 NeuronCores (v2.2: bf16 dataflow).

Problem (per reference):
  X [4, 4096, 1024]; Wq/Wk [1024, 256]; Wv [1024, 128]; biases; lam scalar.
  Q,K = X@Wq+bq, X@Wk+bk ; V = X@Wv+bv
  A_i = Q_i @ K_i^T / sqrt(128)  (i = 1,2 : the two 128-wide halves)
  out = (softmax(A1) - lam * softmax(A2)) @ V          -> [4, 4096, 128]

Sharding: 8 cores = 4 batches x 2 query-halves. Each core computes the
attention output for 2048 queries of one batch; K/V projections for the
full 4096 keys of that batch are computed redundantly on both cores of the
pair (no collectives). Host passes X^T per core (bf16) with the core's
query rows ordered first; key order is irrelevant to softmax.

v2.2 dataflow (all matmul operands bf16, PSUM fp32):
  Projections per chunk-pair (2x512 cols), weight-stationary inner loop,
  double-buffered PSUM so ScalarE bias-evacuations overlap the next
  pair's matmuls. V goes through PE transposes into [key, d] layout.
  Attention in 4 super-chunks of 512 queries x 32 key tiles; per step
  both components' scores land in one [128, 2x512] fp32 PSUM tile
  (double-buffered so the PE runs a full step ahead of ScalarE), one
  FD=1024 exp ACTIVATE (1/sqrt(D) folded into the scale) writes e12
  bf16, DVE accumulates softmax denominators bf16 at 2x, PV accumulates
  in PSUM over the 32 key tiles. Finalize per super: ones-matmul
  broadcasts the denominator partition-sums, DVE evacuates + applies
  reciprocal_approx_fast; lam rides the o2 evacuation scale. Output
  ships as O^T [128, 2048] fp32; host transposes.
"""

import sys

sys.path.insert(0, "/opt/trn_rl_repo")

import numpy as np

import concourse.bacc as bacc
import concourse.mybir as mybir
from concourse import masks
from concourse.tile import TileContext
from concourse.bass_utils import run_bass_kernel_spmd

F32 = mybir.dt.float32
BF16 = mybir.dt.bfloat16
AF = mybir.ActivationFunctionType

D = 128
EMB = 1024
B, S = 4, 4096
NQ = S // 2          # queries per core
SQC = 512            # projection column chunk
NCC = S // SQC       # 8 projection column chunks
NE = EMB // 128      # 8 emb k-tiles
SUP = 512            # attention query super-chunk
NSUP = NQ // SUP     # 4
NSK = S // 128       # 32 key tiles
INV_SQRT_D = 1.0 / np.sqrt(np.float32(D))

TRACE = False
TRACE_DIR = None
LAST_RESULT = None


def _build():
    nc = bacc.Bacc("TRN2", target_bir_lowering=False, debug=False, num_devices=8)

    xt = nc.dram_tensor("xt", [EMB, S], BF16, kind="ExternalInput")
    wq = nc.dram_tensor("wq", [EMB, 2 * D], BF16, kind="ExternalInput")
    wk = nc.dram_tensor("wk", [EMB, 2 * D], BF16, kind="ExternalInput")
    wv = nc.dram_tensor("wv", [EMB, D], BF16, kind="ExternalInput")
    bq = nc.dram_tensor("bq", [2 * D, 1], F32, kind="ExternalInput")
    bk = nc.dram_tensor("bk", [2 * D, 1], F32, kind="ExternalInput")
    bv = nc.dram_tensor("bv", [D, 1], F32, kind="ExternalInput")
    lamv = nc.dram_tensor("lamv", [128, 1], F32, kind="ExternalInput")
    out = nc.dram_tensor("o", [D, NQ], F32, kind="ExternalOutput")  # O^T

    from contextlib import ExitStack

    with TileContext(nc) as tc, ExitStack() as ctx:
        wpool = ctx.enter_context(tc.tile_pool(name="w", bufs=1))
        wk1 = wpool.tile([128, NE, 128], BF16, tag="wk1")
        wk2 = wpool.tile([128, NE, 128], BF16, tag="wk2")
        wq1 = wpool.tile([128, NE, 128], BF16, tag="wq1")
        wq2 = wpool.tile([128, NE, 128], BF16, tag="wq2")
        wvt = wpool.tile([128, NE, 128], BF16, tag="wvt")

        xpool = ctx.enter_context(tc.tile_pool(name="xt", bufs=1))
        xt_all = xpool.tile([128, NCC, NE, SQC], BF16, tag="xall")

        def wsrc(w, dsl):
            return w[:, dsl].rearrange("(t p) d -> p t d", p=128)

        def load_chunk(cc):
            nc.sync.dma_start(
                out=xt_all[:, cc],
                in_=xt[:, cc * SQC : (cc + 1) * SQC].rearrange(
                    "(t p) s -> p t s", p=128
                ),
            )

        nc.sync.dma_start(out=wk1[:], in_=wsrc(wk, slice(0, 128)))
        load_chunk(0)
        nc.sync.dma_start(out=wk2[:], in_=wsrc(wk, slice(128, 256)))
        load_chunk(1)
        nc.sync.dma_start(out=wvt[:], in_=wsrc(wv, slice(0, 128)))
        nc.sync.dma_start(out=wq1[:], in_=wsrc(wq, slice(0, 128)))
        nc.sync.dma_start(out=wq2[:], in_=wsrc(wq, slice(128, 256)))
        for cc in range(2, NCC):
            load_chunk(cc)

        cpool = ctx.enter_context(tc.tile_pool(name="const", bufs=1))
        ident = cpool.tile([128, 128], BF16, tag="ident")
        masks.make_identity(nc, ident[:])
        ones_sq = cpool.tile([128, 128], BF16, tag="ones_sq")
        nc.vector.memset(ones_sq[:], 1.0)

        bq1 = cpool.tile([128, 1], F32, tag="bq1")
        bq2 = cpool.tile([128, 1], F32, tag="bq2")
        bk1 = cpool.tile([128, 1], F32, tag="bk1")
        bk2 = cpool.tile([128, 1], F32, tag="bk2")
        bvt = cpool.tile([128, 1], F32, tag="bvt")
        lam_t = cpool.tile([128, 1], F32, tag="lam")
        nc.gpsimd.dma_start(out=bq1[:], in_=bq[0:128, :])
        nc.gpsimd.dma_start(out=bq2[:], in_=bq[128:256, :])
        nc.gpsimd.dma_start(out=bk1[:], in_=bk[0:128, :])
        nc.gpsimd.dma_start(out=bk2[:], in_=bk[128:256, :])
        nc.gpsimd.dma_start(out=bvt[:], in_=bv[0:128, :])
        nc.gpsimd.dma_start(out=lam_t[:], in_=lamv[:, :])

        qkv = ctx.enter_context(tc.tile_pool(name="qkv", bufs=1))
        qt1 = qkv.tile([128, NQ], BF16, tag="qt1")
        qt2 = qkv.tile([128, NQ], BF16, tag="qt2")
        kt1 = qkv.tile([128, S], BF16, tag="kt1")
        kt2 = qkv.tile([128, S], BF16, tag="kt2")
        vv = qkv.tile([128, S], BF16, tag="vv")  # [key%128, kt*128+d] = V[key, d]

        epool = ctx.enter_context(tc.tile_pool(name="e", bufs=3))
        papool = ctx.enter_context(tc.tile_pool(name="pacc", bufs=2))
        fpool = ctx.enter_context(tc.tile_pool(name="fin", bufs=2))

        # ---------------- projections ----------------
        with ExitStack() as pctx:
            ppool = pctx.enter_context(tc.tile_pool(name="ppsum", bufs=2, space="PSUM"))
            tpool = pctx.enter_context(tc.tile_pool(name="ptr", bufs=2, space="PSUM"))
            vspool = pctx.enter_context(tc.tile_pool(name="vts", bufs=2))

            def proj_job(dst, w_t, b_t, pair, tag, first=False):
                chunks = (2 * pair, 2 * pair + 1)
                ps = ppool.tile([128, 2, SQC], F32, tag="proj", name=f"ps_{tag}")
                if first:
                    # chunk-outer so compute starts as soon as chunk 0 lands
                    for ci in range(2):
                        for e in range(NE):
                            nc.tensor.matmul(
                                ps[:, ci, :],
                                w_t[:, e, :],
                                xt_all[:, chunks[ci], e, :],
                                start=(e == 0),
                                stop=(e == NE - 1),
                            )
                else:
                    # weight-stationary: one LDW feeds both chunks
                    for e in range(NE):
                        for ci in range(2):
                            nc.tensor.matmul(
                                ps[:, ci, :],
                                w_t[:, e, :],
                                xt_all[:, chunks[ci], e, :],
                                start=(e == 0),
                                stop=(e == NE - 1),
                            )
                if dst is not None:
                    csl = slice(chunks[0] * SQC, (chunks[0] + 2) * SQC)
                    nc.scalar.activation(
                        dst[:, csl], ps[:], AF.Identity, bias=b_t[:, 0:1]
                    )
                else:
                    vt_s = vspool.tile(
                        [128, 2, SQC], BF16, tag="vts", name=f"vt_{pair}"
                    )
                    nc.scalar.activation(
                        vt_s[:], ps[:], AF.Identity, bias=b_t[:, 0:1]
                    )
                    for ci in range(2):
                        for j in range(SQC // 128):
                            tr = tpool.tile(
                                [128, 128], BF16, tag="vtr",
                                name=f"vtr_{pair}_{ci}_{j}",
                            )
                            nc.tensor.transpose(
                                tr[:], vt_s[:, ci, j * 128 : (j + 1) * 128], ident[:]
                            )
                            col = (chunks[ci] * (SQC // 128) + j) * 128
                            nc.vector.tensor_copy(vv[:, col : col + 128], tr[:])

            for pair in range(NCC // 2):
                proj_job(kt1, wk1, bk1, pair, f"k1_{pair}", first=(pair == 0))
                proj_job(kt2, wk2, bk2, pair, f"k2_{pair}")
                proj_job(None, wvt, bvt, pair, f"v_{pair}")
                if pair < 2:
                    proj_job(qt1, wq1, bq1, pair, f"q1_{pair}")
                    proj_job(qt2, wq2, bq2, pair, f"q2_{pair}")

        # ---------------- attention ----------------
        with ExitStack() as actx:
            spool = actx.enter_context(tc.tile_pool(name="spsum", bufs=2, space="PSUM"))
            rpool = actx.enter_context(tc.tile_pool(name="rpsum", bufs=1, space="PSUM"))
            opool = actx.enter_context(tc.tile_pool(name="opsum", bufs=1, space="PSUM"))

            for sup in range(NSUP):
                qa = sup * SUP
                o1 = opool.tile([128, SUP], F32, tag="o1", name=f"o1_{sup}")
                o2 = opool.tile([128, SUP], F32, tag="o2", name=f"o2_{sup}")
                pacc = papool.tile([128, 2, SUP], BF16, tag="pacc", name=f"pa_{sup}")

                for kt in range(NSK):
                    ksl = slice(kt * 128, (kt + 1) * 128)
                    s12 = spool.tile(
                        [128, 2, SUP], F32, tag="s12", name=f"s12_{sup}_{kt}"
                    )
                    nc.tensor.matmul(
                        s12[:, 0, :], kt1[:, ksl], qt1[:, qa : qa + SUP],
                        start=True, stop=True,
                    )
                    nc.tensor.matmul(
                        s12[:, 1, :], kt2[:, ksl], qt2[:, qa : qa + SUP],
                        start=True, stop=True,
                    )
                    e12 = epool.tile(
                        [128, 2, SUP], BF16, tag="e12", name=f"e_{sup}_{kt}"
                    )
                    nc.scalar.activation(
                        e12[:], s12[:], AF.Exp, scale=float(INV_SQRT_D)
                    )
                    if kt == 0:
                        nc.vector.tensor_copy(pacc[:], e12[:])
                    else:
                        nc.vector.tensor_add(pacc[:], pacc[:], e12[:])
                    nc.tensor.matmul(
                        o1[:], vv[:, ksl], e12[:, 0, :],
                        start=(kt == 0), stop=(kt == NSK - 1),
                    )
                    nc.tensor.matmul(
                        o2[:], vv[:, ksl], e12[:, 1, :],
                        start=(kt == 0), stop=(kt == NSK - 1),
                    )

                # ---- finalize this super-chunk ----
                rs_b = rpool.tile([128, 2, SUP], F32, tag="rsb", name=f"rsb_{sup}")
                for comp in range(2):
                    nc.tensor.matmul(
                        rs_b[:, comp, :], ones_sq[:], pacc[:, comp, :],
                        start=True, stop=True,
                    )
                rsr = fpool.tile([128, 2, SUP], F32, tag="rsr", name=f"rsr_{sup}")
                nc.vector.tensor_copy(rsr[:], rs_b[:])
                o1s = fpool.tile([128, SUP], BF16, tag="o1s", name=f"o1s_{sup}")
                nc.vector.tensor_copy(o1s[:], o1[:])
                # fold lam into the o2 evacuation (ScalarE: scale rides the copy)
                o2s = fpool.tile([128, SUP], BF16, tag="o2s", name=f"o2s_{sup}")
                nc.scalar.activation(o2s[:], o2[:], AF.Identity, scale=lam_t[:, 0:1])

                ir = fpool.tile([128, 2, SUP], F32, tag="ir", name=f"ir_{sup}")
                nc.vector.reciprocal_approx_fast(ir[:], rsr[:])
                t1 = fpool.tile([128, SUP], F32, tag="t1", name=f"t1_{sup}")
                nc.vector.tensor_mul(t1[:], o1s[:], ir[:, 0, :])
                t2 = fpool.tile([128, SUP], F32, tag="t2", name=f"t2_{sup}")
                nc.vector.tensor_mul(t2[:], o2s[:], ir[:, 1, :])
                o_t = fpool.tile([128, SUP], F32, tag="ot", name=f"ot_{sup}")
                nc.vector.tensor_sub(o_t[:], t1[:], t2[:])
                nc.sync.dma_start(out=out[:, qa : qa + SUP], in_=o_t[:])

    nc.compile()
    return nc


_NC = None


def _get_nc():
    global _NC
    if _NC is None:
        _NC = _build()
    return _NC


def kernel(X, lam, Wq, bq, Wk, bk, Wv, bv):
    import ml_dtypes

    BF = ml_dtypes.bfloat16
    X = np.asarray(X, dtype=np.float32)
    lam_f = float(np.asarray(lam))
    Wq_b = np.ascontiguousarray(np.asarray(Wq, np.float32).astype(BF))
    Wk_b = np.ascontiguousarray(np.asarray(Wk, np.float32).astype(BF))
    Wv_b = np.ascontiguousarray(np.asarray(Wv, np.float32).astype(BF))
    bq_c = np.asarray(bq, np.float32).reshape(2 * D, 1).copy()
    bk_c = np.asarray(bk, np.float32).reshape(2 * D, 1).copy()
    bv_c = np.asarray(bv, np.float32).reshape(D, 1).copy()
    lam_v = np.full((128, 1), lam_f, np.float32)

    nc = _get_nc()

    in_maps = []
    for core in range(8):
        b, h = divmod(core, 2)
        xb = X[b]
        if h == 0:
            xr = xb
        else:
            xr = np.concatenate([xb[NQ:], xb[:NQ]], axis=0)
        xt_a = np.ascontiguousarray(xr.T.astype(BF))
        in_maps.append(
            {
                "xt": xt_a,
                "wq": Wq_b,
                "wk": Wk_b,
                "wv": Wv_b,
                "bq": bq_c,
                "bk": bk_c,
                "bv": bv_c,
                "lamv": lam_v,
            }
        )

    global LAST_RESULT
    kwargs = {}
    if TRACE:
        import tempfile

        tdir = tempfile.mkdtemp(dir=TRACE_DIR) if TRACE_DIR else None
        kwargs = dict(trace=True, tmpdir=tdir)
    res = run_bass_kernel_spmd(nc, in_maps, list(range(8)), **kwargs)
    LAST_RESULT = res

    o = np.empty((B, S, D), np.float32)
    for core in range(8):
        b, h = divmod(core, 2)
        o[b, h * NQ : (h + 1) * NQ, :] = np.asarray(
            res.results[core]["o"], np.float32
        ).T
    return o


# revision 5
# speedup vs baseline: 1.9410x; 1.0004x over previous
"""DiffAttn kernel for 8 trn2 NeuronCores (v2.2: bf16 dataflow).

Problem (per reference):
  X [4, 4096, 1024]; Wq/Wk [1024, 256]; Wv [1024, 128]; biases; lam scalar.
  Q,K = X@Wq+bq, X@Wk+bk ; V = X@Wv+bv
  A_i = Q_i @ K_i^T / sqrt(128)  (i = 1,2 : the two 128-wide halves)
  out = (softmax(A1) - lam * softmax(A2)) @ V          -> [4, 4096, 128]

Sharding: 8 cores = 4 batches x 2 query-halves. Each core computes the
attention output for 2048 queries of one batch; K/V projections for the
full 4096 keys of that batch are computed redundantly on both cores of the
pair (no collectives). Host passes X^T per core (bf16) with the core's
query rows ordered first; key order is irrelevant to softmax.

v2.2 dataflow (all matmul operands bf16, PSUM fp32):
  Projections per chunk-pair (2x512 cols), weight-stationary inner loop,
  double-buffered PSUM so ScalarE bias-evacuations overlap the next
  pair's matmuls. V goes through PE transposes into [key, d] layout.
  Attention in 4 super-chunks of 512 queries x 32 key tiles; per step
  both components' scores land in one [128, 2x512] fp32 PSUM tile
  (double-buffered so the PE runs a full step ahead of ScalarE), one
  FD=1024 exp ACTIVATE (1/sqrt(D) folded into the scale) writes e12
  bf16, DVE accumulates softmax denominators bf16 at 2x, PV accumulates
  in PSUM over the 32 key tiles. Finalize per super: ones-matmul
  broadcasts the denominator partition-sums, DVE evacuates + applies
  reciprocal_approx_fast; lam rides the o2 evacuation scale. Output
  ships as O^T [128, 2048] fp32; host transposes.
"""

import sys

sys.path.insert(0, "/opt/trn_rl_repo")

import numpy as np

import concourse.bacc as bacc
import concourse.mybir as mybir
from concourse import masks
from concourse.tile import TileContext
from concourse.bass_utils import run_bass_kernel_spmd

F32 = mybir.dt.float32
BF16 = mybir.dt.bfloat16
AF = mybir.ActivationFunctionType

D = 128
EMB = 1024
B, S = 4, 4096
NQ = S // 2          # queries per core
SQC = 512            # projection column chunk
NCC = S // SQC       # 8 projection column chunks
NE = EMB // 128      # 8 emb k-tiles
SUP = 512            # attention query super-chunk
NSUP = NQ // SUP     # 4
NSK = S // 128       # 32 key tiles
INV_SQRT_D = 1.0 / np.sqrt(np.float32(D))

TRACE = False
TRACE_DIR = None
LAST_RESULT = None


def _build():
    nc = bacc.Bacc("TRN2", target_bir_lowering=False, debug=False, num_devices=8)

    xt = nc.dram_tensor("xt", [EMB, S], BF16, kind="ExternalInput")
    wq = nc.dram_tensor("wq", [EMB, 2 * D], BF16, kind="ExternalInput")
    wk = nc.dram_tensor("wk", [EMB, 2 * D], BF16, kind="ExternalInput")
    wv = nc.dram_tensor("wv", [EMB, D], BF16, kind="ExternalInput")
    bq = nc.dram_tensor("bq", [2 * D, 1], F32, kind="ExternalInput")
    bk = nc.dram_tensor("bk", [2 * D, 1], F32, kind="ExternalInput")
    bv = nc.dram_tensor("bv", [D, 1], F32, kind="ExternalInput")
    lamv = nc.dram_tensor("lamv", [128, 1], F32, kind="ExternalInput")
    out = nc.dram_tensor("o", [D, NQ], F32, kind="ExternalOutput")  # O^T

    from contextlib import ExitStack

    with TileContext(nc) as tc, ExitStack() as ctx:
        wpool = ctx.enter_context(tc.tile_pool(name="w", bufs=1))
        wk1 = wpool.tile([128, NE, 128], BF16, tag="wk1")
        wk2 = wpool.tile([128, NE, 128], BF16, tag="wk2")
        wq1 = wpool.tile([128, NE, 128], BF16, tag="wq1")
        wq2 = wpool.tile([128, NE, 128], BF16, tag="wq2")
        wvt = wpool.tile([128, NE, 128], BF16, tag="wvt")

        xpool = ctx.enter_context(tc.tile_pool(name="xt", bufs=1))
        xt_all = xpool.tile([128, NCC, NE, SQC], BF16, tag="xall")

        def wsrc(w, dsl):
            return w[:, dsl].rearrange("(t p) d -> p t d", p=128)

        def load_chunk(cc):
            nc.sync.dma_start(
                out=xt_all[:, cc],
                in_=xt[:, cc * SQC : (cc + 1) * SQC].rearrange(
                    "(t p) s -> p t s", p=128
                ),
            )

        def load_chunk_split(cc):
            # per-e-tile plain 2D slabs: first matmul can start after 1/8
            csl = slice(cc * SQC, (cc + 1) * SQC)
            for e in range(NE):
                nc.sync.dma_start(
                    out=xt_all[:, cc, e, :],
                    in_=xt[e * 128 : (e + 1) * 128, csl],
                )

        nc.sync.dma_start(out=wk1[:], in_=wsrc(wk, slice(0, 128)))
        load_chunk_split(0)
        nc.sync.dma_start(out=wk2[:], in_=wsrc(wk, slice(128, 256)))
        load_chunk_split(1)
        nc.sync.dma_start(out=wvt[:], in_=wsrc(wv, slice(0, 128)))
        nc.sync.dma_start(out=wq1[:], in_=wsrc(wq, slice(0, 128)))
        nc.sync.dma_start(out=wq2[:], in_=wsrc(wq, slice(128, 256)))
        for cc in range(2, NCC):
            load_chunk(cc)

        cpool = ctx.enter_context(tc.tile_pool(name="const", bufs=1))
        ident = cpool.tile([128, 128], BF16, tag="ident")
        masks.make_identity(nc, ident[:])
        ones_sq = cpool.tile([128, 128], BF16, tag="ones_sq")
        nc.vector.memset(ones_sq[:], 1.0)

        bq1 = cpool.tile([128, 1], F32, tag="bq1")
        bq2 = cpool.tile([128, 1], F32, tag="bq2")
        bk1 = cpool.tile([128, 1], F32, tag="bk1")
        bk2 = cpool.tile([128, 1], F32, tag="bk2")
        bvt = cpool.tile([128, 1], F32, tag="bvt")
        lam_t = cpool.tile([128, 1], F32, tag="lam")
        nc.gpsimd.dma_start(out=bq1[:], in_=bq[0:128, :])
        nc.gpsimd.dma_start(out=bq2[:], in_=bq[128:256, :])
        nc.gpsimd.dma_start(out=bk1[:], in_=bk[0:128, :])
        nc.gpsimd.dma_start(out=bk2[:], in_=bk[128:256, :])
        nc.gpsimd.dma_start(out=bvt[:], in_=bv[0:128, :])
        nc.gpsimd.dma_start(out=lam_t[:], in_=lamv[:, :])

        qkv = ctx.enter_context(tc.tile_pool(name="qkv", bufs=1))
        qt1 = qkv.tile([128, NQ], BF16, tag="qt1")
        qt2 = qkv.tile([128, NQ], BF16, tag="qt2")
        kt1 = qkv.tile([128, S], BF16, tag="kt1")
        kt2 = qkv.tile([128, S], BF16, tag="kt2")
        vv = qkv.tile([128, S], BF16, tag="vv")  # [key%128, kt*128+d] = V[key, d]

        epool = ctx.enter_context(tc.tile_pool(name="e", bufs=3))
        papool = ctx.enter_context(tc.tile_pool(name="pacc", bufs=2))
        fpool = ctx.enter_context(tc.tile_pool(name="fin", bufs=2))

        # ---------------- projections ----------------
        with ExitStack() as pctx:
            ppool = pctx.enter_context(tc.tile_pool(name="ppsum", bufs=2, space="PSUM"))
            tpool = pctx.enter_context(tc.tile_pool(name="ptr", bufs=2, space="PSUM"))
            vspool = pctx.enter_context(tc.tile_pool(name="vts", bufs=2))

            def proj_job(dst, w_t, b_t, pair, tag, first=False):
                chunks = (2 * pair, 2 * pair + 1)
                ps = ppool.tile([128, 2, SQC], F32, tag="proj", name=f"ps_{tag}")
                if first:
                    # chunk-outer so compute starts as soon as chunk 0 lands
                    for ci in range(2):
                        for e in range(NE):
                            nc.tensor.matmul(
                                ps[:, ci, :],
                                w_t[:, e, :],
                                xt_all[:, chunks[ci], e, :],
                                start=(e == 0),
                                stop=(e == NE - 1),
                            )
                else:
                    # weight-stationary: one LDW feeds both chunks
                    for e in range(NE):
                        for ci in range(2):
                            nc.tensor.matmul(
                                ps[:, ci, :],
                                w_t[:, e, :],
                                xt_all[:, chunks[ci], e, :],
                                start=(e == 0),
                                stop=(e == NE - 1),
                            )
                if dst is not None:
                    csl = slice(chunks[0] * SQC, (chunks[0] + 2) * SQC)
                    nc.scalar.activation(
                        dst[:, csl], ps[:], AF.Identity, bias=b_t[:, 0:1]
                    )
                else:
                    vt_s = vspool.tile(
                        [128, 2, SQC], BF16, tag="vts", name=f"vt_{pair}"
                    )
                    nc.scalar.activation(
                        vt_s[:], ps[:], AF.Identity, bias=b_t[:, 0:1]
                    )
                    for ci in range(2):
                        for j in range(SQC // 128):
                            tr = tpool.tile(
                                [128, 128], BF16, tag="vtr",
                                name=f"vtr_{pair}_{ci}_{j}",
                            )
                            nc.tensor.transpose(
                                tr[:], vt_s[:, ci, j * 128 : (j + 1) * 128], ident[:]
                            )
                            col = (chunks[ci] * (SQC // 128) + j) * 128
                            nc.vector.tensor_copy(vv[:, col : col + 128], tr[:])

            for pair in range(NCC // 2):
                proj_job(kt1, wk1, bk1, pair, f"k1_{pair}", first=(pair == 0))
                proj_job(kt2, wk2, bk2, pair, f"k2_{pair}")
                proj_job(None, wvt, bvt, pair, f"v_{pair}")
                if pair < 2:
                    proj_job(qt1, wq1, bq1, pair, f"q1_{pair}")
                    proj_job(qt2, wq2, bq2, pair, f"q2_{pair}")

        # ---------------- attention ----------------
        with ExitStack() as actx:
            spool = actx.enter_context(tc.tile_pool(name="spsum", bufs=2, space="PSUM"))
            rpool = actx.enter_context(tc.tile_pool(name="rpsum", bufs=1, space="PSUM"))
            opool = actx.enter_context(tc.tile_pool(name="opsum", bufs=1, space="PSUM"))

            for sup in range(NSUP):
                qa = sup * SUP
                o1 = opool.tile([128, SUP], F32, tag="o1", name=f"o1_{sup}")
                o2 = opool.tile([128, SUP], F32, tag="o2", name=f"o2_{sup}")
                pacc = papool.tile([128, 2, SUP], BF16, tag="pacc", name=f"pa_{sup}")

                for kt in range(NSK):
                    ksl = slice(kt * 128, (kt + 1) * 128)
                    s12 = spool.tile(
                        [128, 2, SUP], F32, tag="s12", name=f"s12_{sup}_{kt}"
                    )
                    nc.tensor.matmul(
                        s12[:, 0, :], kt1[:, ksl], qt1[:, qa : qa + SUP],
                        start=True, stop=True,
                    )
                    nc.tensor.matmul(
                        s12[:, 1, :], kt2[:, ksl], qt2[:, qa : qa + SUP],
                        start=True, stop=True,
                    )
                    e12 = epool.tile(
                        [128, 2, SUP], BF16, tag="e12", name=f"e_{sup}_{kt}"
                    )
                    nc.scalar.activation(
                        e12[:], s12[:], AF.Exp, scale=float(INV_SQRT_D)
                    )
                    if kt == 0:
                        nc.vector.tensor_copy(pacc[:], e12[:])
                    else:
                        nc.vector.tensor_add(pacc[:], pacc[:], e12[:])
                    nc.tensor.matmul(
                        o1[:], vv[:, ksl], e12[:, 0, :],
                        start=(kt == 0), stop=(kt == NSK - 1),
                    )
                    nc.tensor.matmul(
                        o2[:], vv[:, ksl], e12[:, 1, :],
                        start=(kt == 0), stop=(kt == NSK - 1),
                    )

                # ---- finalize this super-chunk ----
                rs_b = rpool.tile([128, 2, SUP], F32, tag="rsb", name=f"rsb_{sup}")
                for comp in range(2):
                    nc.tensor.matmul(
                        rs_b[:, comp, :], ones_sq[:], pacc[:, comp, :],
                        start=True, stop=True,
                    )
                # o evacuations on ScalarE (parallel with DVE's reciprocal)
                o1s = fpool.tile([128, SUP], BF16, tag="o1s", name=f"o1s_{sup}")
                nc.scalar.activation(o1s[:], o1[:], AF.Identity)
                # fold lam into the o2 evacuation (ScalarE: scale rides the copy)
                o2s = fpool.tile([128, SUP], BF16, tag="o2s", name=f"o2s_{sup}")
                nc.scalar.activation(o2s[:], o2[:], AF.Identity, scale=lam_t[:, 0:1])

                ir = fpool.tile([128, 2, SUP], F32, tag="ir", name=f"ir_{sup}")
                nc.vector.reciprocal_approx_fast(ir[:], rs_b[:])
                t1 = fpool.tile([128, SUP], F32, tag="t1", name=f"t1_{sup}")
                nc.vector.tensor_mul(t1[:], o1s[:], ir[:, 0, :])
                t2 = fpool.tile([128, SUP], F32, tag="t2", name=f"t2_{sup}")
                nc.vector.tensor_mul(t2[:], o2s[:], ir[:, 1, :])
                o_t = fpool.tile([128, SUP], F32, tag="ot", name=f"ot_{sup}")
                nc.vector.tensor_sub(o_t[:], t1[:], t2[:])
                nc.sync.dma_start(out=out[:, qa : qa + SUP], in_=o_t[:])

    nc.compile()
    return nc


_NC = None


def _get_nc():
    global _NC
    if _NC is None:
        _NC = _build()
    return _NC


def kernel(X, lam, Wq, bq, Wk, bk, Wv, bv):
    import ml_dtypes

    BF = ml_dtypes.bfloat16
    X = np.asarray(X, dtype=np.float32)
    lam_f = float(np.asarray(lam))
    Wq_b = np.ascontiguousarray(np.asarray(Wq, np.float32).astype(BF))
    Wk_b = np.ascontiguousarray(np.asarray(Wk, np.float32).astype(BF))
    Wv_b = np.ascontiguousarray(np.asarray(Wv, np.float32).astype(BF))
    bq_c = np.asarray(bq, np.float32).reshape(2 * D, 1).copy()
    bk_c = np.asarray(bk, np.float32).reshape(2 * D, 1).copy()
    bv_c = np.asarray(bv, np.float32).reshape(D, 1).copy()
    lam_v = np.full((128, 1), lam_f, np.float32)

    nc = _get_nc()

    in_maps = []
    for core in range(8):
        b, h = divmod(core, 2)
        xb = X[b]
        if h == 0:
            xr = xb
        else:
            xr = np.concatenate([xb[NQ:], xb[:NQ]], axis=0)
        xt_a = np.ascontiguousarray(xr.T.astype(BF))
        in_maps.append(
            {
                "xt": xt_a,
                "wq": Wq_b,
                "wk": Wk_b,
                "wv": Wv_b,
                "bq": bq_c,
                "bk": bk_c,
                "bv": bv_c,
                "lamv": lam_v,
            }
        )

    global LAST_RESULT
    kwargs = {}
    if TRACE:
        import tempfile

        tdir = tempfile.mkdtemp(dir=TRACE_DIR) if TRACE_DIR else None
        kwargs = dict(trace=True, tmpdir=tdir)
    res = run_bass_kernel_spmd(nc, in_maps, list(range(8)), **kwargs)
    LAST_RESULT = res

    o = np.empty((B, S, D), np.float32)
    for core in range(8):
        b, h = divmod(core, 2)
        o[b, h * NQ : (h + 1) * NQ, :] = np.asarray(
            res.results[core]["o"], np.float32
        ).T
    return o


# revision 6
# speedup vs baseline: 1.9425x; 1.0008x over previous
"""DiffAttn kernel for 8 trn2 NeuronCores (v2.4: bf16, proj/attention interleave).

Problem (per reference):
  X [4, 4096, 1024]; Wq/Wk [1024, 256]; Wv [1024, 128]; biases; lam scalar.
  Q,K = X@Wq+bq, X@Wk+bk ; V = X@Wv+bv
  A_i = Q_i @ K_i^T / sqrt(128)  (i = 1,2 : the two 128-wide halves)
  out = (softmax(A1) - lam * softmax(A2)) @ V          -> [4, 4096, 128]

Sharding: 8 cores = 4 batches x 2 query-halves. Each core computes the
attention output for 2048 queries of one batch; K/V projections for the
full 4096 keys of that batch are computed redundantly on both cores of
the pair. Host passes X^T per core (bf16), its query rows first.

v2.4 dataflow (all matmul operands bf16, PSUM fp32):
  ScalarE's exp stream is the pacer (~1.2us per 128x1024 step), so the
  PE's projection work is interleaved INTO the attention step stream as
  single-chunk jobs to keep the PE from idling (HAM downclocks an idle
  PE). Attention runs in 4 supers of 512 queries, split into 16-key-tile
  half-windows scheduled (s0,h0),(s1,h0),(s0,h1),(s1,h1),(s2,h0)... so
  the PV PSUM accumulators only span one window (2 banks instead of 4)
  with DVE merging partials in SBUF; freed banks hold the projection
  PSUM. Per step: 2 scores matmuls into a double-buffered [128, 2x512]
  fp32 PSUM tile, one FD=1024 exp ACTIVATE -> e12 bf16, DVE accumulates
  softmax denominators bf16 at 2x, 2 PV matmuls. V reaches its [key, d]
  layout via DMA-xbar transposes (no PE/PSUM). Finalize per super:
  ones-matmul broadcasts the denominator partition-sums into a borrowed
  scores buffer, DVE reciprocal_approx_fast + 3 tensor ops; lam folds
  into the reciprocal. Output ships as O^T [128, 2048] fp32.
"""

import sys

sys.path.insert(0, "/opt/trn_rl_repo")

import numpy as np

import concourse.bacc as bacc
import concourse.mybir as mybir
from concourse.tile import TileContext
from concourse.bass_utils import run_bass_kernel_spmd

F32 = mybir.dt.float32
BF16 = mybir.dt.bfloat16
AF = mybir.ActivationFunctionType

D = 128
EMB = 1024
B, S = 4, 4096
NQ = S // 2          # queries per core
SQC = 512            # projection column chunk
NCC = S // SQC       # 8 projection column chunks
NE = EMB // 128      # 8 emb k-tiles
SUP = 512            # attention query super-chunk
NSUP = NQ // SUP     # 4
NSK = S // 128       # 32 key tiles
HKT = 16             # key tiles per segment (half window)
INV_SQRT_D = 1.0 / np.sqrt(np.float32(D))

# segment schedule: supers in pairs, half-windows alternated so PV PSUM
# only ever holds one window and chunk c isn't needed before step 8*c
SEGS = [(0, 0), (1, 0), (0, 1), (1, 1), (2, 0), (3, 0), (2, 1), (3, 1)]

TRACE = False
TRACE_DIR = None
LAST_RESULT = None


def _build():
    nc = bacc.Bacc("TRN2", target_bir_lowering=False, debug=False, num_devices=8)

    xt = nc.dram_tensor("xt", [EMB, S], BF16, kind="ExternalInput")
    wq = nc.dram_tensor("wq", [EMB, 2 * D], BF16, kind="ExternalInput")
    wk = nc.dram_tensor("wk", [EMB, 2 * D], BF16, kind="ExternalInput")
    wv = nc.dram_tensor("wv", [EMB, D], BF16, kind="ExternalInput")
    bq = nc.dram_tensor("bq", [2 * D, 1], F32, kind="ExternalInput")
    bk = nc.dram_tensor("bk", [2 * D, 1], F32, kind="ExternalInput")
    bv = nc.dram_tensor("bv", [D, 1], F32, kind="ExternalInput")
    lamv = nc.dram_tensor("lamv", [128, 1], F32, kind="ExternalInput")
    out = nc.dram_tensor("o", [D, NQ], F32, kind="ExternalOutput")  # O^T

    from contextlib import ExitStack

    with TileContext(nc) as tc, ExitStack() as ctx:
        wpool = ctx.enter_context(tc.tile_pool(name="w", bufs=1))
        wk1 = wpool.tile([128, NE, 128], BF16, tag="wk1")
        wk2 = wpool.tile([128, NE, 128], BF16, tag="wk2")
        wq1 = wpool.tile([128, NE, 128], BF16, tag="wq1")
        wq2 = wpool.tile([128, NE, 128], BF16, tag="wq2")
        wvt = wpool.tile([128, NE, 128], BF16, tag="wvt")

        xpool = ctx.enter_context(tc.tile_pool(name="xt", bufs=1))
        xt_all = xpool.tile([128, NCC, NE, SQC], BF16, tag="xall")

        def wsrc(w, dsl):
            return w[:, dsl].rearrange("(t p) d -> p t d", p=128)

        def load_chunk(cc):
            nc.sync.dma_start(
                out=xt_all[:, cc],
                in_=xt[:, cc * SQC : (cc + 1) * SQC].rearrange(
                    "(t p) s -> p t s", p=128
                ),
            )

        def load_chunk_split(cc):
            csl = slice(cc * SQC, (cc + 1) * SQC)
            for e in range(NE):
                nc.sync.dma_start(
                    out=xt_all[:, cc, e, :],
                    in_=xt[e * 128 : (e + 1) * 128, csl],
                )

        nc.sync.dma_start(out=wk1[:], in_=wsrc(wk, slice(0, 128)))
        load_chunk_split(0)
        nc.sync.dma_start(out=wk2[:], in_=wsrc(wk, slice(128, 256)))
        load_chunk_split(1)
        nc.sync.dma_start(out=wvt[:], in_=wsrc(wv, slice(0, 128)))
        nc.sync.dma_start(out=wq1[:], in_=wsrc(wq, slice(0, 128)))
        nc.sync.dma_start(out=wq2[:], in_=wsrc(wq, slice(128, 256)))
        for cc in range(2, NCC):
            load_chunk(cc)

        cpool = ctx.enter_context(tc.tile_pool(name="const", bufs=1))
        ones_sq = cpool.tile([128, 128], BF16, tag="ones_sq")
        nc.vector.memset(ones_sq[:], 1.0)

        bq1 = cpool.tile([128, 1], F32, tag="bq1")
        bq2 = cpool.tile([128, 1], F32, tag="bq2")
        bk1 = cpool.tile([128, 1], F32, tag="bk1")
        bk2 = cpool.tile([128, 1], F32, tag="bk2")
        bvt = cpool.tile([128, 1], F32, tag="bvt")
        lam_t = cpool.tile([128, 1], F32, tag="lam")
        nc.gpsimd.dma_start(out=bq1[:], in_=bq[0:128, :])
        nc.gpsimd.dma_start(out=bq2[:], in_=bq[128:256, :])
        nc.gpsimd.dma_start(out=bk1[:], in_=bk[0:128, :])
        nc.gpsimd.dma_start(out=bk2[:], in_=bk[128:256, :])
        nc.gpsimd.dma_start(out=bvt[:], in_=bv[0:128, :])
        nc.gpsimd.dma_start(out=lam_t[:], in_=lamv[:, :])

        qkv = ctx.enter_context(tc.tile_pool(name="qkv", bufs=1))
        qt1 = qkv.tile([128, NQ], BF16, tag="qt1")
        qt2 = qkv.tile([128, NQ], BF16, tag="qt2")
        kt1 = qkv.tile([128, S], BF16, tag="kt1")
        kt2 = qkv.tile([128, S], BF16, tag="kt2")
        vv = qkv.tile([128, S], BF16, tag="vv")  # [key%128, kt*128+d] = V[key, d]

        epool = ctx.enter_context(tc.tile_pool(name="e", bufs=3))
        papool = ctx.enter_context(tc.tile_pool(name="pacc", bufs=3))
        fpool = ctx.enter_context(tc.tile_pool(name="fin", bufs=2))
        vspool = ctx.enter_context(tc.tile_pool(name="vts", bufs=2))

        # PSUM: spool 2x2 banks + opool 2 banks + ppool 2 banks = 8 banks
        spool = ctx.enter_context(tc.tile_pool(name="spsum", bufs=2, space="PSUM"))
        opool = ctx.enter_context(tc.tile_pool(name="opsum", bufs=1, space="PSUM"))
        ppool = ctx.enter_context(tc.tile_pool(name="ppsum", bufs=2, space="PSUM"))

        # ---------------- projection jobs (single chunk) ----------------
        def proj_job(dst, w_t, b_t, c, tag):
            ps = ppool.tile([128, SQC], F32, tag="pp", name=f"ps_{tag}")
            for e in range(NE):
                nc.tensor.matmul(
                    ps[:],
                    w_t[:, e, :],
                    xt_all[:, c, e, :],
                    start=(e == 0),
                    stop=(e == NE - 1),
                )
            csl = slice(c * SQC, (c + 1) * SQC)
            if dst is not None:
                nc.scalar.activation(
                    dst[:, csl], ps[:], AF.Identity, bias=b_t[:, 0:1]
                )
            else:
                vt_s = vspool.tile([128, SQC], BF16, tag="vts", name=f"vt_{c}")
                nc.scalar.activation(
                    vt_s[:], ps[:], AF.Identity, bias=b_t[:, 0:1]
                )
                for j in range(SQC // 128):
                    col = (c * (SQC // 128) + j) * 128
                    nc.sync.dma_start_transpose(
                        vv[:, col : col + 128], vt_s[:, j * 128 : (j + 1) * 128]
                    )

        def jk1(c):
            return lambda: proj_job(kt1, wk1, bk1, c, f"k1_{c}")

        def jk2(c):
            return lambda: proj_job(kt2, wk2, bk2, c, f"k2_{c}")

        def jv(c):
            return lambda: proj_job(None, wvt, bvt, c, f"v_{c}")

        def jq1(c):
            return lambda: proj_job(qt1, wq1, bq1, c, f"q1_{c}")

        def jq2(c):
            return lambda: proj_job(qt2, wq2, bq2, c, f"q2_{c}")

        # lead: chunk 0 fully projected before attention starts
        for job in [jk1(0), jk2(0), jv(0), jq1(0), jq2(0)]:
            job()

        # interleave schedule: global step -> proj jobs to emit first.
        # kv chunk c must be emitted before step 8*(c%4) of its window
        # era (c1..c3 in steps 0..11, c4..c7 before steps 32..44);
        # q chunk c before the first segment of super c (16, 64, 80).
        due = {
            0: [jk1(1)], 1: [jk2(1)], 2: [jv(1)],
            4: [jk1(2)], 5: [jk2(2)], 6: [jv(2)],
            8: [jk1(3)], 9: [jk2(3)], 10: [jv(3)],
            12: [jq1(1)], 14: [jq2(1)],
            16: [jk1(4)], 18: [jk2(4)], 20: [jv(4)],
            22: [jk1(5)], 24: [jk2(5)], 26: [jv(5)],
            28: [jk1(6)], 30: [jk2(6)], 32: [jv(6)],
            34: [jk1(7)], 36: [jk2(7)], 38: [jv(7)],
            44: [jq1(2)], 50: [jq2(2)],
            56: [jq1(3)], 62: [jq2(3)],
        }

        # ---------------- attention ----------------
        state = {}
        gstep = 0
        for sup, h in SEGS:
            qa = sup * SUP
            o1 = opool.tile([128, SUP], F32, tag="o1", name=f"o1_{sup}_{h}")
            o2 = opool.tile([128, SUP], F32, tag="o2", name=f"o2_{sup}_{h}")
            if h == 0:
                state[sup] = {
                    "pacc": papool.tile(
                        [128, 2, SUP], BF16, tag="pacc", name=f"pa_{sup}"
                    )
                }
            st = state[sup]
            pacc = st["pacc"]

            for i in range(HKT):
                for job in due.get(gstep, ()):
                    job()
                kt = h * HKT + i
                ksl = slice(kt * 128, (kt + 1) * 128)
                s12 = spool.tile(
                    [128, 2, SUP], F32, tag="s12", name=f"s12_{sup}_{kt}"
                )
                nc.tensor.matmul(
                    s12[:, 0, :], kt1[:, ksl], qt1[:, qa : qa + SUP],
                    start=True, stop=True,
                )
                nc.tensor.matmul(
                    s12[:, 1, :], kt2[:, ksl], qt2[:, qa : qa + SUP],
                    start=True, stop=True,
                )
                e12 = epool.tile(
                    [128, 2, SUP], BF16, tag="e12", name=f"e_{sup}_{kt}"
                )
                nc.scalar.activation(
                    e12[:], s12[:], AF.Exp, scale=float(INV_SQRT_D)
                )
                if kt == 0:
                    nc.vector.tensor_copy(pacc[:], e12[:])
                else:
                    nc.vector.tensor_add(pacc[:], pacc[:], e12[:])
                nc.tensor.matmul(
                    o1[:], vv[:, ksl], e12[:, 0, :],
                    start=(i == 0), stop=(i == HKT - 1),
                )
                nc.tensor.matmul(
                    o2[:], vv[:, ksl], e12[:, 1, :],
                    start=(i == 0), stop=(i == HKT - 1),
                )
                gstep += 1

            # ---- segment end: move PV partials to SBUF ----
            if h == 0:
                po1 = fpool.tile([128, SUP], F32, tag="po1", name=f"po1_{sup}")
                nc.vector.tensor_copy(po1[:], o1[:])
                po2 = fpool.tile([128, SUP], F32, tag="po2", name=f"po2_{sup}")
                nc.vector.tensor_copy(po2[:], o2[:])
                st["po1"], st["po2"] = po1, po2
            else:
                po1, po2 = st["po1"], st["po2"]
                nc.vector.tensor_add(po1[:], po1[:], o1[:])
                nc.vector.tensor_add(po2[:], po2[:], o2[:])

                # ---- finalize this super ----
                rs_b = spool.tile(
                    [128, 2, SUP], F32, tag="s12", name=f"rsb_{sup}"
                )
                for comp in range(2):
                    nc.tensor.matmul(
                        rs_b[:, comp, :], ones_sq[:], pacc[:, comp, :],
                        start=True, stop=True,
                    )
                ir = fpool.tile([128, 2, SUP], F32, tag="ir", name=f"ir_{sup}")
                nc.vector.reciprocal_approx_fast(ir[:], rs_b[:])
                # fold lam into the comp-2 reciprocal
                nc.vector.tensor_scalar_mul(
                    ir[:, 1, :], ir[:, 1, :], lam_t[:, 0:1]
                )
                t1 = fpool.tile([128, SUP], F32, tag="t1", name=f"t1_{sup}")
                nc.vector.tensor_mul(t1[:], po1[:], ir[:, 0, :])
                t2 = fpool.tile([128, SUP], F32, tag="t2", name=f"t2_{sup}")
                nc.vector.tensor_mul(t2[:], po2[:], ir[:, 1, :])
                o_t = fpool.tile([128, SUP], F32, tag="ot", name=f"ot_{sup}")
                nc.vector.tensor_sub(o_t[:], t1[:], t2[:])
                nc.sync.dma_start(out=out[:, qa : qa + SUP], in_=o_t[:])
                del state[sup]

    nc.compile()
    return nc


_NC = None


def _get_nc():
    global _NC
    if _NC is None:
        _NC = _build()
    return _NC


def kernel(X, lam, Wq, bq, Wk, bk, Wv, bv):
    import ml_dtypes

    BF = ml_dtypes.bfloat16
    X = np.asarray(X, dtype=np.float32)
    lam_f = float(np.asarray(lam))
    Wq_b = np.ascontiguousarray(np.asarray(Wq, np.float32).astype(BF))
    Wk_b = np.ascontiguousarray(np.asarray(Wk, np.float32).astype(BF))
    Wv_b = np.ascontiguousarray(np.asarray(Wv, np.float32).astype(BF))
    bq_c = np.asarray(bq, np.float32).reshape(2 * D, 1).copy()
    bk_c = np.asarray(bk, np.float32).reshape(2 * D, 1).copy()
    bv_c = np.asarray(bv, np.float32).reshape(D, 1).copy()
    lam_v = np.full((128, 1), lam_f, np.float32)

    nc = _get_nc()

    in_maps = []
    for core in range(8):
        b, h = divmod(core, 2)
        xb = X[b]
        if h == 0:
            xr = xb
        else:
            xr = np.concatenate([xb[NQ:], xb[:NQ]], axis=0)
        xt_a = np.ascontiguousarray(xr.T.astype(BF))
        in_maps.append(
            {
                "xt": xt_a,
                "wq": Wq_b,
                "wk": Wk_b,
                "wv": Wv_b,
                "bq": bq_c,
                "bk": bk_c,
                "bv": bv_c,
                "lamv": lam_v,
            }
        )

    global LAST_RESULT
    kwargs = {}
    if TRACE:
        import tempfile

        tdir = tempfile.mkdtemp(dir=TRACE_DIR) if TRACE_DIR else None
        kwargs = dict(trace=True, tmpdir=tdir)
    res = run_bass_kernel_spmd(nc, in_maps, list(range(8)), **kwargs)
    LAST_RESULT = res

    o = np.empty((B, S, D), np.float32)
    for core in range(8):
        b, h = divmod(core, 2)
        o[b, h * NQ : (h + 1) * NQ, :] = np.asarray(
            res.results[core]["o"], np.float32
        ).T
    return o
